# revision 1
# baseline (speedup 1.0000x reference)
"""Trainium2 Bass kernel for nn_Coefficients: assemble the sparse circuit
coefficient matrix

    out = [ kcl  = [ M | 0 ]                       (N rows)
            kvl  = [ 0 | I_E | -M^T ]              (E rows)
            elem = diag(z) / diag(y) scatter ]     (E rows)

Row-wise shard across 8 NeuronCores: core d produces
  - kcl:  M[d*256:(d+1)*256, :]            (DRAM->DRAM copy)
  - mt:   -M[:, d*512:(d+1)*512]^T         (PE transpose + negate)
  - eye:  I bands (512x128), zb/yb: diag(z)/diag(y) bands computed from
          params/kinds on device.
The host unshards: places each core's blocks/bands at their row/column
offsets in the zero canvas (pure indexing — all numeric content is
device-produced).

The m_cols load trick: a flat [2048,512] DRAM block reshaped to SBUF
[128, 2048] quarters keeps every DMA descriptor 8KB-contiguous; the
resulting n = 16*p + 4*jg + jj interleave is undone for free in the
PSUM->SBUF copy's strided access pattern.
"""

import numpy as np

N = 2048
E = 4096
W = 2 * E + N  # 10240
D = 8
NR = N // D  # 256 kcl rows per core
EC = E // D  # 512 kvl/elem rows per core

_CACHE: dict = {}


def _build(opts=None):
    import concourse.bacc as bacc
    import concourse.tile as tile
    import concourse.mybir as mybir
    from concourse._compat import get_trn_type

    opts = dict(opts or {})
    kcl_on_gpsimd = opts.get("kcl_on_gpsimd", False)
    tpool_bufs = opts.get("tpool_bufs", 2)
    ppool_bufs = opts.get("ppool_bufs", 8)

    f32 = mybir.dt.float32
    i32 = mybir.dt.int32

    nc = bacc.Bacc(
        get_trn_type() or "TRN2",
        target_bir_lowering=False,
        debug=False,
        enable_asserts=False,
        num_devices=D,
    )

    m_rows = nc.dram_tensor("m_rows", [NR, E], f32, kind="ExternalInput")
    m_cols = nc.dram_tensor("m_cols", [N, EC], f32, kind="ExternalInput")
    params_s = nc.dram_tensor("params_s", [128, 4], f32, kind="ExternalInput")
    kinds_s = nc.dram_tensor("kinds_s", [128, 4], i32, kind="ExternalInput")

    kcl = nc.dram_tensor("kcl", [NR, E], f32, kind="ExternalOutput")
    mt = nc.dram_tensor("mt", [EC, N], f32, kind="ExternalOutput")
    # bands in SBUF-friendly layouts (fully contiguous single DMAs); the host
    # reindexes: eye block is identical for all 4 chunks, zb/yb are [p, (c q)]
    eye = nc.dram_tensor("eye", [128, 128], f32, kind="ExternalOutput")
    zb = nc.dram_tensor("zb", [128, EC], f32, kind="ExternalOutput")
    yb = nc.dram_tensor("yb", [128, EC], f32, kind="ExternalOutput")

    AO = mybir.AluOpType

    # m_cols flat view: element (n, e) lives at flat n*512+e; SBUF quarter jg
    # holds partitions p with contiguous 8KB runs: n = 16p + 4*jg + jj.
    mflat = m_cols.ap().rearrange("n e -> (n e)").rearrange(
        "(p q f) -> p q f", p=128, q=4
    )  # [p, jg, 2048] with per-(p,jg) contiguous 2048 f32

    with tile.TileContext(nc) as tc:
        with (
            tc.tile_pool(name="cpool", bufs=1) as cpool,
            tc.tile_pool(name="tpool", bufs=tpool_bufs) as tpool,
            tc.tile_pool(name="ppool", bufs=ppool_bufs, space="PSUM") as ppool,
        ):
            # ---- m_cols quarters first on both HWDGE rings so the PE can
            # start ASAP: [128, 2048], 8KB contiguous per partition
            mcq = []
            for jg in range(4):
                t = cpool.tile([128, 2048], f32, tag=f"mc{jg}")
                eng = nc.sync if jg % 2 == 0 else nc.scalar
                eng.dma_start(out=t[:], in_=mflat[:, jg, :])
                mcq.append(t)

            # ---- small inputs
            pt = cpool.tile([128, 4], f32)
            kt = cpool.tile([128, 4], f32)
            nc.sync.dma_start(out=pt[:], in_=params_s.ap()[:, :])
            nc.gpsimd.dma_start(out=kt[:], in_=kinds_s.ap()[:, :])  # i32 -> f32

            # ---- kcl M block: DRAM -> DRAM, no dependents
            if kcl_on_gpsimd:
                nc.gpsimd.dma_start(out=kcl.ap()[:, :], in_=m_rows.ap()[:, :])
            else:
                nc.sync.dma_start(
                    out=kcl.ap()[0 : NR // 2, :], in_=m_rows.ap()[0 : NR // 2, :]
                )
                nc.scalar.dma_start(
                    out=kcl.ap()[NR // 2 : NR, :], in_=m_rows.ap()[NR // 2 : NR, :]
                )

            # ---- identity tile (also the eye-band payload)
            ident = cpool.tile([128, 128], f32)
            nc.gpsimd.memset(ident[:], 0.0)
            nc.gpsimd.affine_select(
                out=ident[:],
                in_=ident[:],
                compare_op=AO.not_equal,
                fill=1.0,
                base=0,
                pattern=[[-1, 128]],
                channel_multiplier=1,
            )

            # ---- z/y diagonal values (layout r = c*128 + p)
            rm = cpool.tile([128, 4], f32)
            im = cpool.tile([128, 4], f32)
            vm = cpool.tile([128, 4], f32)
            sm = cpool.tile([128, 4], f32)
            onm = cpool.tile([128, 4], f32)
            offm = cpool.tile([128, 4], f32)
            zv = cpool.tile([128, 4], f32)
            yv = cpool.tile([128, 4], f32)
            t0 = cpool.tile([128, 4], f32)
            t1 = cpool.tile([128, 4], f32)

            nc.vector.tensor_scalar(rm[:], kt[:], 0.0, None, op0=AO.is_equal)
            nc.vector.tensor_scalar(im[:], kt[:], 1.0, None, op0=AO.is_equal)
            nc.vector.tensor_scalar(vm[:], kt[:], 2.0, None, op0=AO.is_equal)
            nc.vector.tensor_scalar(sm[:], kt[:], 3.0, None, op0=AO.is_equal)
            nc.vector.tensor_scalar(onm[:], pt[:], 0.0, None, op0=AO.is_gt)
            nc.vector.tensor_scalar(offm[:], pt[:], 0.0, None, op0=AO.is_le)
            # z = vc + sw*off - r*params
            nc.vector.tensor_tensor(t0[:], sm[:], offm[:], op=AO.mult)
            nc.vector.tensor_tensor(t0[:], vm[:], t0[:], op=AO.add)
            nc.vector.tensor_tensor(t1[:], rm[:], pt[:], op=AO.mult)
            nc.vector.tensor_tensor(zv[:], t0[:], t1[:], op=AO.subtract)
            # y = r + ivs + sw*on
            nc.vector.tensor_tensor(t0[:], sm[:], onm[:], op=AO.mult)
            nc.vector.tensor_tensor(t0[:], im[:], t0[:], op=AO.add)
            nc.vector.tensor_tensor(yv[:], rm[:], t0[:], op=AO.add)

            # ---- diagonal bands: all 4 chunks built side by side, then one
            # contiguous DMA per tensor on the HWDGE rings (no SWDGE tail)
            zd_all = cpool.tile([128, EC], f32)
            yd_all = cpool.tile([128, EC], f32)
            for c in range(4):
                nc.vector.tensor_scalar(
                    zd_all[:, c * 128 : (c + 1) * 128], ident[:], zv[:, c : c + 1],
                    None, op0=AO.mult,
                )
                nc.vector.tensor_scalar(
                    yd_all[:, c * 128 : (c + 1) * 128], ident[:], yv[:, c : c + 1],
                    None, op0=AO.mult,
                )
            # gpsimd queue is otherwise idle and these are contiguous-descriptor
            # writes, so they land mid-kernel instead of extending the tail
            nc.gpsimd.dma_start(out=eye.ap()[:, :], in_=ident[:])
            nc.gpsimd.dma_start(out=zb.ap()[:, :], in_=zd_all[:])
            nc.gpsimd.dma_start(out=yb.ap()[:, :], in_=yd_all[:])

            # ---- -M^T: PE transpose, n = 16p + 4jg + jj undone in copy APs
            for ec in range(4):
                T = tpool.tile([128, N], f32, tag="T")
                # dst view [e, j(16), p2(128)]: free index = p2*16 + j
                Tv = T[:].rearrange("e (p2 j) -> e j p2", j=16)
                for jg in range(4):
                    ps = ppool.tile([128, 512], f32)
                    for jj in range(4):
                        src = mcq[jg][:, jj * 512 + ec * 128 : jj * 512 + ec * 128 + 128]
                        nc.tensor.transpose(
                            out=ps[:, jj * 128 : (jj + 1) * 128],
                            in_=src,
                            identity=ident[:],
                        )
                    # negate + un-interleave: T[e, 16*p2 + 4*jg + jj] = -ps[e, jj*128+p2]
                    # alternate DVE / ACT so neither engine paces the PE
                    dst = Tv[:, 4 * jg : 4 * jg + 4, :]
                    src = ps[:].rearrange("e (jj p2) -> e jj p2", p2=128)
                    if (ec * 4 + jg) % 2 == 0:
                        nc.vector.tensor_scalar(dst, src, -1.0, None, op0=AO.mult)
                    else:
                        nc.scalar.activation(
                            dst, src, mybir.ActivationFunctionType.Copy, scale=-1.0
                        )
                eng = nc.sync if ec % 2 == 0 else nc.scalar
                eng.dma_start(out=mt.ap()[ec * 128 : (ec + 1) * 128, :], in_=T[:])

    nc.compile()
    return nc


def _get_nc(opts=None):
    key = ("nc", tuple(sorted((opts or {}).items())))
    if key not in _CACHE:
        _CACHE[key] = _build(opts)
    return _CACHE[key]


def _in_maps(M, params, kinds):
    maps = []
    for d in range(D):
        maps.append(
            {
                "m_rows": np.ascontiguousarray(M[d * NR : (d + 1) * NR, :]),
                "m_cols": np.ascontiguousarray(M[:, d * EC : (d + 1) * EC]),
                "params_s": np.ascontiguousarray(
                    params[d * EC : (d + 1) * EC].reshape(4, 128).T
                ),
                "kinds_s": np.ascontiguousarray(
                    kinds[d * EC : (d + 1) * EC].reshape(4, 128).T
                ),
            }
        )
    return maps


def kernel(M, params, kinds, _trace=False, _trace_kwargs=None, _opts=None):
    from concourse.bass_utils import run_bass_kernel_spmd

    M = np.ascontiguousarray(np.asarray(M, dtype=np.float32))
    params = np.ascontiguousarray(np.asarray(params, dtype=np.float32))
    kinds = np.ascontiguousarray(np.asarray(kinds, dtype=np.int32))
    assert M.shape == (N, E) and params.shape == (E,) and kinds.shape == (E,)

    nc = _get_nc(_opts)
    res = run_bass_kernel_spmd(
        nc,
        _in_maps(M, params, kinds),
        core_ids=list(range(D)),
        trace=_trace,
        **(_trace_kwargs or {}),
    )
    out = np.zeros((N + 2 * E, W), np.float32)
    for d in range(D):
        r = res.results[d]
        out[d * NR : (d + 1) * NR, 0:E] = r["kcl"]
        out[N + d * EC : N + (d + 1) * EC, 2 * E :] = r["mt"]
        zb3 = r["zb"].reshape(128, 4, 128)
        yb3 = r["yb"].reshape(128, 4, 128)
        for c in range(4):
            g0 = d * EC + c * 128  # global elem index of band start
            out[N + g0 : N + g0 + 128, E + g0 : E + g0 + 128] = r["eye"]
            out[N + E + g0 : N + E + g0 + 128, g0 : g0 + 128] = zb3[:, c, :]
            out[N + E + g0 : N + E + g0 + 128, E + g0 : E + g0 + 128] = yb3[:, c, :]
    if _trace:
        _CACHE["last_result"] = res
    return out



# revision 3
# speedup vs baseline: 1.3252x; 1.3252x over previous
"""Trainium2 Bass kernel for nn_Coefficients: assemble the sparse circuit
coefficient matrix

    out = [ kcl  = [ M | 0 ]                       (N rows)
            kvl  = [ 0 | I_E | -M^T ]              (E rows)
            elem = diag(z) / diag(y) scatter ]     (E rows)

Row-wise shard of M across 8 NeuronCores: core d loads its 256-row shard
M[d*256:(d+1)*256, :] from HBM ONCE and derives both output blocks from it:
  - kcl:  the shard itself, cast to fp16 (SBUF->DRAM)
  - mt:   -shard^T via PE transpose = the 256-COLUMN slice
          [4096, 256] of -M^T (column-sharded kvl right block)
  - eye / zb / yb: tiny diagonal bands from params/kinds, as before.
This cuts per-core HBM traffic from 16 MiB (baseline: shard read twice +
two f32 writes) to ~8.6 MiB (one f32 read + two fp16 writes), which is the
binding constraint at the ~358 GB/s per-core HBM limit.

fp16 carries 11 significand bits -> max rel err ~4.9e-4 on the M-derived
blocks, well inside the 2e-2 gate; the host widens fp16->f32 during
placement (an exact cast). Bands stay f32/exact.

The host unshards by pure indexing: each core's blocks land at their
row/column offsets in the zero canvas; mt arrives in the SBUF-friendly
[128, 32*256] layout ([q, (c j)]) and is un-interleaved with a reshape/
transpose (all numeric content is device-produced).
"""

import numpy as np

N = 2048
E = 4096
W = 2 * E + N  # 10240
D = 8
NR = N // D  # 256 kcl rows / mt columns per core
EC = E // D  # 512 band elems per core

_CACHE: dict = {}


def _build(opts=None):
    import concourse.bacc as bacc
    import concourse.tile as tile
    import concourse.mybir as mybir
    from concourse._compat import get_trn_type

    opts = dict(opts or {})
    half_out = opts.get("half_out", True)  # fp16 kcl/mt outputs
    ppool_bufs = opts.get("ppool_bufs", 8)

    f32 = mybir.dt.float32
    f16 = mybir.dt.float16
    i32 = mybir.dt.int32
    odt = f16 if half_out else f32

    nc = bacc.Bacc(
        get_trn_type() or "TRN2",
        target_bir_lowering=False,
        debug=False,
        enable_asserts=False,
        num_devices=D,
    )

    m_rows = nc.dram_tensor("m_rows", [NR, E], f32, kind="ExternalInput")
    params_s = nc.dram_tensor("params_s", [128, 4], f32, kind="ExternalInput")
    kinds_s = nc.dram_tensor("kinds_s", [128, 4], i32, kind="ExternalInput")

    kcl = nc.dram_tensor("kcl", [NR, E], odt, kind="ExternalOutput")
    # mt layout [q, (c j)]: mt[q, c*256+j] = -M[d*256+j, c*128+q]; host
    # reshape(128,32,256).transpose(1,0,2).reshape(4096,256) -> -M^T cols
    mt = nc.dram_tensor("mt", [128, 32 * NR], odt, kind="ExternalOutput")
    # bands in SBUF-friendly layouts (contiguous single DMAs); host
    # reindexes: eye block is identical for all 4 chunks, zb/yb are [p, (c q)]
    eye = nc.dram_tensor("eye", [128, 128], f32, kind="ExternalOutput")
    zb = nc.dram_tensor("zb", [128, EC], f32, kind="ExternalOutput")
    yb = nc.dram_tensor("yb", [128, EC], f32, kind="ExternalOutput")

    AO = mybir.AluOpType
    ACT_COPY = mybir.ActivationFunctionType.Copy

    with tile.TileContext(nc) as tc:
        with (
            tc.tile_pool(name="cpool", bufs=1) as cpool,
            tc.tile_pool(name="ppool", bufs=ppool_bufs, space="PSUM") as ppool,
        ):
            # ---- shard loads, 1 MiB chunks on both HWDGE rings
            in0 = cpool.tile([128, E], f32, tag="in0")  # shard rows 0..127
            in1 = cpool.tile([128, E], f32, tag="in1")  # shard rows 128..255
            H = E // 2
            nc.sync.dma_start(out=in0[:, 0:H], in_=m_rows.ap()[0:128, 0:H])
            nc.scalar.dma_start(out=in1[:, 0:H], in_=m_rows.ap()[128:256, 0:H])
            nc.sync.dma_start(out=in0[:, H:E], in_=m_rows.ap()[0:128, H:E])
            nc.scalar.dma_start(out=in1[:, H:E], in_=m_rows.ap()[128:256, H:E])

            # ---- small inputs (SWDGE; kinds cast i32 -> f32 during DMA)
            pt = cpool.tile([128, 4], f32)
            kt = cpool.tile([128, 4], f32)
            nc.gpsimd.dma_start(out=pt[:], in_=params_s.ap()[:, :])
            nc.gpsimd.dma_start(out=kt[:], in_=kinds_s.ap()[:, :])

            # ---- identity tile (f32: eye-band payload; odt copy for the PE)
            ident = cpool.tile([128, 128], f32)
            nc.gpsimd.memset(ident[:], 0.0)
            nc.gpsimd.affine_select(
                out=ident[:],
                in_=ident[:],
                compare_op=AO.not_equal,
                fill=1.0,
                base=0,
                pattern=[[-1, 128]],
                channel_multiplier=1,
            )
            nc.gpsimd.dma_start(out=eye.ap()[:, :], in_=ident[:])

            if half_out:
                ident_o = cpool.tile([128, 128], odt, tag="ident_o")
                nc.vector.tensor_copy(ident_o[:], ident[:])
            else:
                ident_o = ident

            # ---- shard cast f32 -> odt (DVE), chunk-matched to the loads
            if half_out:
                h0 = cpool.tile([128, E], odt, tag="h0")
                h1 = cpool.tile([128, E], odt, tag="h1")
                nc.vector.tensor_copy(h0[:, 0:H], in0[:, 0:H])
                nc.vector.tensor_copy(h1[:, 0:H], in1[:, 0:H])
                nc.vector.tensor_copy(h0[:, H:E], in0[:, H:E])
                nc.vector.tensor_copy(h1[:, H:E], in1[:, H:E])
            else:
                h0, h1 = in0, in1

            # ---- kcl stores from the cast shard (0.5 MiB chunks)
            nc.sync.dma_start(out=kcl.ap()[0:128, 0:H], in_=h0[:, 0:H])
            nc.scalar.dma_start(out=kcl.ap()[128:256, 0:H], in_=h1[:, 0:H])
            nc.sync.dma_start(out=kcl.ap()[0:128, H:E], in_=h0[:, H:E])
            nc.scalar.dma_start(out=kcl.ap()[128:256, H:E], in_=h1[:, H:E])

            # ---- -M^T: 32 col-chunks x 2 row-halves of PE transpose;
            # negate folded into the PSUM->SBUF copy (DVE/ACT alternate)
            stg = [
                cpool.tile([128, 8 * NR], odt, name=f"stg{g}", tag=f"stg{g}")
                for g in range(4)
            ]
            for c in range(32):
                g, s = c // 8, (c % 8) * NR
                ps = ppool.tile([128, NR], odt)
                nc.tensor.transpose(
                    out=ps[:, 0:128],
                    in_=h0[:, c * 128 : (c + 1) * 128],
                    identity=ident_o[:],
                )
                nc.tensor.transpose(
                    out=ps[:, 128:256],
                    in_=h1[:, c * 128 : (c + 1) * 128],
                    identity=ident_o[:],
                )
                dst = stg[g][:, s : s + NR]
                if c % 2 == 0:
                    nc.vector.tensor_scalar(dst, ps[:], -1.0, None, op0=AO.mult)
                else:
                    nc.scalar.activation(dst, ps[:], ACT_COPY, scale=-1.0)
                if c % 8 == 7:
                    eng = nc.sync if g % 2 == 0 else nc.scalar
                    eng.dma_start(
                        out=mt.ap()[:, g * 8 * NR : (g + 1) * 8 * NR], in_=stg[g][:]
                    )

            # ---- z/y diagonal values (layout r = c*128 + p)
            rm = cpool.tile([128, 4], f32)
            im = cpool.tile([128, 4], f32)
            vm = cpool.tile([128, 4], f32)
            sm = cpool.tile([128, 4], f32)
            onm = cpool.tile([128, 4], f32)
            offm = cpool.tile([128, 4], f32)
            zv = cpool.tile([128, 4], f32)
            yv = cpool.tile([128, 4], f32)
            t0 = cpool.tile([128, 4], f32)
            t1 = cpool.tile([128, 4], f32)

            nc.vector.tensor_scalar(rm[:], kt[:], 0.0, None, op0=AO.is_equal)
            nc.vector.tensor_scalar(im[:], kt[:], 1.0, None, op0=AO.is_equal)
            nc.vector.tensor_scalar(vm[:], kt[:], 2.0, None, op0=AO.is_equal)
            nc.vector.tensor_scalar(sm[:], kt[:], 3.0, None, op0=AO.is_equal)
            nc.vector.tensor_scalar(onm[:], pt[:], 0.0, None, op0=AO.is_gt)
            nc.vector.tensor_scalar(offm[:], pt[:], 0.0, None, op0=AO.is_le)
            # z = vc + sw*off - r*params
            nc.vector.tensor_tensor(t0[:], sm[:], offm[:], op=AO.mult)
            nc.vector.tensor_tensor(t0[:], vm[:], t0[:], op=AO.add)
            nc.vector.tensor_tensor(t1[:], rm[:], pt[:], op=AO.mult)
            nc.vector.tensor_tensor(zv[:], t0[:], t1[:], op=AO.subtract)
            # y = r + ivs + sw*on
            nc.vector.tensor_tensor(t0[:], sm[:], onm[:], op=AO.mult)
            nc.vector.tensor_tensor(t0[:], im[:], t0[:], op=AO.add)
            nc.vector.tensor_tensor(yv[:], rm[:], t0[:], op=AO.add)

            # ---- diagonal bands: 4 chunks side by side, one DMA per tensor
            zd_all = cpool.tile([128, EC], f32)
            yd_all = cpool.tile([128, EC], f32)
            for c in range(4):
                nc.vector.tensor_scalar(
                    zd_all[:, c * 128 : (c + 1) * 128], ident[:], zv[:, c : c + 1],
                    None, op0=AO.mult,
                )
                nc.vector.tensor_scalar(
                    yd_all[:, c * 128 : (c + 1) * 128], ident[:], yv[:, c : c + 1],
                    None, op0=AO.mult,
                )
            nc.gpsimd.dma_start(out=zb.ap()[:, :], in_=zd_all[:])
            nc.gpsimd.dma_start(out=yb.ap()[:, :], in_=yd_all[:])

    nc.compile()
    return nc


def _get_nc(opts=None):
    key = ("nc", tuple(sorted((opts or {}).items())))
    if key not in _CACHE:
        _CACHE[key] = _build(opts)
    return _CACHE[key]


def _in_maps(M, params, kinds):
    maps = []
    for d in range(D):
        maps.append(
            {
                "m_rows": np.ascontiguousarray(M[d * NR : (d + 1) * NR, :]),
                "params_s": np.ascontiguousarray(
                    params[d * EC : (d + 1) * EC].reshape(4, 128).T
                ),
                "kinds_s": np.ascontiguousarray(
                    kinds[d * EC : (d + 1) * EC].reshape(4, 128).T
                ),
            }
        )
    return maps


def kernel(M, params, kinds, _trace=False, _trace_kwargs=None, _opts=None):
    from concourse.bass_utils import run_bass_kernel_spmd

    M = np.ascontiguousarray(np.asarray(M, dtype=np.float32))
    params = np.ascontiguousarray(np.asarray(params, dtype=np.float32))
    kinds = np.ascontiguousarray(np.asarray(kinds, dtype=np.int32))
    assert M.shape == (N, E) and params.shape == (E,) and kinds.shape == (E,)

    nc = _get_nc(_opts)
    res = run_bass_kernel_spmd(
        nc,
        _in_maps(M, params, kinds),
        core_ids=list(range(D)),
        trace=_trace,
        **(_trace_kwargs or {}),
    )
    out = np.zeros((N + 2 * E, W), np.float32)
    for d in range(D):
        r = res.results[d]
        out[d * NR : (d + 1) * NR, 0:E] = r["kcl"]
        out[N : N + E, 2 * E + d * NR : 2 * E + (d + 1) * NR] = (
            r["mt"].reshape(128, 32, NR).transpose(1, 0, 2).reshape(E, NR)
        )
        zb3 = r["zb"].reshape(128, 4, 128)
        yb3 = r["yb"].reshape(128, 4, 128)
        for c in range(4):
            g0 = d * EC + c * 128  # global elem index of band start
            out[N + g0 : N + g0 + 128, E + g0 : E + g0 + 128] = r["eye"]
            out[N + E + g0 : N + E + g0 + 128, g0 : g0 + 128] = zb3[:, c, :]
            out[N + E + g0 : N + E + g0 + 128, E + g0 : E + g0 + 128] = yb3[:, c, :]
    if _trace:
        _CACHE["last_result"] = res
    return out


# revision 7
# speedup vs baseline: 1.5638x; 1.1801x over previous
"""Trainium2 Bass kernel for nn_Coefficients: assemble the sparse circuit
coefficient matrix

    out = [ kcl  = [ M | 0 ]                       (N rows)
            kvl  = [ 0 | I_E | -M^T ]              (E rows)
            elem = diag(z) / diag(y) scatter ]     (E rows)

Row-wise shard of M across 8 NeuronCores: core d loads its 256-row shard
M[d*256:(d+1)*256, :] from HBM ONCE and derives both output blocks from it:
  - kcl:  the shard itself, cast to fp16 (SBUF->DRAM)
  - mt:   -shard^T via PE transpose = the 256-COLUMN slice
          [4096, 256] of -M^T (column-sharded kvl right block)
  - bands: eye / diag(z) / diag(y) from params/kinds, fused in one store.
This cuts per-core HBM traffic from 16 MiB (baseline: shard read twice +
two f32 writes) to ~8.3 MiB (one f32 read + fp16 writes), the binding
constraint at the ~358 GB/s per-core HBM limit.  fp16 carries 11
significand bits -> max rel err ~4.9e-4 on the value-carrying blocks,
well inside the 2e-2 gate; the host widens fp16->f32 during placement
(an exact cast).

Layout notes: small params/kinds loads go on the HWDGE rings FIRST (a
SWDGE load queued behind the megabyte loads arrives ~10us late and
stalls DVE); no gpsimd DMAs at all.  Instruction/semaphore count is kept
low (teardown sem-sweep costs ~100ns/sem/engine): 4 big loads, 4 kcl
stores, 4 mt stores, 1 band store, 8 fused [128,1024] PSUM->SBUF copies.
The host unshards by pure indexing (mt arrives as [q, (c j)] and is
un-interleaved with reshape/transpose; all numeric content is
device-produced).
"""

import numpy as np

N = 2048
E = 4096
W = 2 * E + N  # 10240
D = 8
NR = N // D  # 256 kcl rows / mt columns per core
EC = E // D  # 512 band elems per core

_CACHE: dict = {}


def _build(opts=None):
    import concourse.bacc as bacc
    import concourse.tile as tile
    import concourse.mybir as mybir
    from concourse._compat import get_trn_type

    opts = dict(opts or {})
    ppool_bufs = opts.get("ppool_bufs", 4)

    f32 = mybir.dt.float32
    f16 = mybir.dt.float16
    i32 = mybir.dt.int32

    nc = bacc.Bacc(
        get_trn_type() or "TRN2",
        target_bir_lowering=False,
        debug=False,
        enable_asserts=False,
        num_devices=D,
    )

    m_rows = nc.dram_tensor("m_rows", [NR, E], f32, kind="ExternalInput")
    params_s = nc.dram_tensor("params_s", [128, 4], f32, kind="ExternalInput")
    kinds_s = nc.dram_tensor("kinds_s", [128, 4], i32, kind="ExternalInput")

    kcl = nc.dram_tensor("kcl", [NR, E], f16, kind="ExternalOutput")
    # mt layout [q, (c j)]: mt[q, c*256+j] = -M[d*256+j, c*128+q]; host
    # reshape(128,32,256).transpose(1,0,2).reshape(4096,256) -> -M^T cols
    mt = nc.dram_tensor("mt", [128, 32 * NR], f16, kind="ExternalOutput")
    # bands [128, 1152]: [0:128] eye, [128:640] diag(z), [640:1152] diag(y),
    # each as 4 side-by-side [128,128] chunks (elem index = c*128 + p)
    bands = nc.dram_tensor("bands", [128, 1152], f16, kind="ExternalOutput")

    AO = mybir.AluOpType
    ACT_COPY = mybir.ActivationFunctionType.Copy
    H = E // 2

    with tile.TileContext(nc) as tc:
        with (
            tc.tile_pool(name="cpool", bufs=1) as cpool,
            tc.tile_pool(name="ppool", bufs=ppool_bufs, space="PSUM") as ppool,
        ):
            # ---- tiny inputs first on the HWDGE rings (SWDGE behind the
            # big loads would arrive ~10us late and stall DVE)
            pt = cpool.tile([128, 4], f32)
            kti = cpool.tile([128, 4], i32)
            nc.sync.dma_start(out=pt[:], in_=params_s.ap()[:, :])
            nc.scalar.dma_start(out=kti[:], in_=kinds_s.ap()[:, :])

            # ---- shard loads, 1 MiB chunks on both HWDGE rings
            in0 = cpool.tile([128, E], f32, tag="in0")  # shard rows 0..127
            in1 = cpool.tile([128, E], f32, tag="in1")  # shard rows 128..255
            nc.sync.dma_start(out=in0[:, 0:H], in_=m_rows.ap()[0:128, 0:H])
            nc.scalar.dma_start(out=in1[:, 0:H], in_=m_rows.ap()[128:256, 0:H])
            nc.sync.dma_start(out=in0[:, H:E], in_=m_rows.ap()[0:128, H:E])
            nc.scalar.dma_start(out=in1[:, H:E], in_=m_rows.ap()[128:256, H:E])

            # ---- fp16 identity (PE transpose operand + eye-band payload)
            ident = cpool.tile([128, 128], f16)
            nc.gpsimd.memset(ident[:], 0.0)
            nc.gpsimd.affine_select(
                out=ident[:],
                in_=ident[:],
                compare_op=AO.not_equal,
                fill=1.0,
                base=0,
                pattern=[[-1, 128]],
                channel_multiplier=1,
            )

            # ---- shard cast f32 -> fp16 (DVE), chunk-matched to the loads
            h0 = cpool.tile([128, E], f16, tag="h0")
            h1 = cpool.tile([128, E], f16, tag="h1")
            nc.vector.tensor_copy(h0[:, 0:H], in0[:, 0:H])
            nc.vector.tensor_copy(h1[:, 0:H], in1[:, 0:H])
            nc.vector.tensor_copy(h0[:, H:E], in0[:, H:E])
            nc.vector.tensor_copy(h1[:, H:E], in1[:, H:E])

            # ---- kcl stores from the cast shard (0.5 MiB chunks)
            nc.sync.dma_start(out=kcl.ap()[0:128, 0:H], in_=h0[:, 0:H])
            nc.scalar.dma_start(out=kcl.ap()[128:256, 0:H], in_=h1[:, 0:H])
            nc.sync.dma_start(out=kcl.ap()[0:128, H:E], in_=h0[:, H:E])
            nc.scalar.dma_start(out=kcl.ap()[128:256, H:E], in_=h1[:, H:E])

            # ---- -M^T: 32 col-chunks x 2 row-halves of PE transpose into
            # [128,1024] PSUM banks (4 chunks each); negate folded into the
            # fused PSUM->SBUF copies (DVE/ACT alternate)
            stg = [
                cpool.tile([128, 8 * NR], f16, name=f"stg{t}", tag=f"stg{t}")
                for t in range(4)
            ]
            for q in range(8):
                ps = ppool.tile([128, 1024], f16)
                for k in range(4):
                    c = 4 * q + k
                    nc.tensor.transpose(
                        out=ps[:, k * 256 : k * 256 + 128],
                        in_=h0[:, c * 128 : (c + 1) * 128],
                        identity=ident[:],
                    )
                    nc.tensor.transpose(
                        out=ps[:, k * 256 + 128 : (k + 1) * 256],
                        in_=h1[:, c * 128 : (c + 1) * 128],
                        identity=ident[:],
                    )
                dst = stg[q // 2][:, (q % 2) * 1024 : (q % 2 + 1) * 1024]
                if q % 2 == 0:
                    nc.vector.tensor_scalar(dst, ps[:], -1.0, None, op0=AO.mult)
                else:
                    nc.scalar.activation(dst, ps[:], ACT_COPY, scale=-1.0)
                if q % 2 == 1:
                    t = q // 2
                    eng = nc.sync if t % 2 == 0 else nc.scalar
                    eng.dma_start(
                        out=mt.ap()[:, t * 2048 : (t + 1) * 2048], in_=stg[t][:]
                    )

            # ---- z/y diagonal values (layout r = c*128 + p)
            ktf = cpool.tile([128, 4], f32)
            rm = cpool.tile([128, 4], f32)
            im = cpool.tile([128, 4], f32)
            vm = cpool.tile([128, 4], f32)
            sm = cpool.tile([128, 4], f32)
            onm = cpool.tile([128, 4], f32)
            offm = cpool.tile([128, 4], f32)
            zv = cpool.tile([128, 4], f32)
            yv = cpool.tile([128, 4], f32)
            t0 = cpool.tile([128, 4], f32)
            t1 = cpool.tile([128, 4], f32)
            nc.vector.tensor_copy(ktf[:], kti[:])
            nc.vector.tensor_scalar(rm[:], ktf[:], 0.0, None, op0=AO.is_equal)
            nc.vector.tensor_scalar(im[:], ktf[:], 1.0, None, op0=AO.is_equal)
            nc.vector.tensor_scalar(vm[:], ktf[:], 2.0, None, op0=AO.is_equal)
            nc.vector.tensor_scalar(sm[:], ktf[:], 3.0, None, op0=AO.is_equal)
            nc.vector.tensor_scalar(onm[:], pt[:], 0.0, None, op0=AO.is_gt)
            nc.vector.tensor_scalar(offm[:], pt[:], 0.0, None, op0=AO.is_le)
            # z = vc + sw*off - r*params
            nc.vector.tensor_tensor(t0[:], sm[:], offm[:], op=AO.mult)
            nc.vector.tensor_tensor(t0[:], vm[:], t0[:], op=AO.add)
            nc.vector.tensor_tensor(t1[:], rm[:], pt[:], op=AO.mult)
            nc.vector.tensor_tensor(zv[:], t0[:], t1[:], op=AO.subtract)
            # y = r + ivs + sw*on
            nc.vector.tensor_tensor(t0[:], sm[:], onm[:], op=AO.mult)
            nc.vector.tensor_tensor(t0[:], im[:], t0[:], op=AO.add)
            nc.vector.tensor_tensor(yv[:], rm[:], t0[:], op=AO.add)

            # ---- band assembly: [eye | diag(z) chunks | diag(y) chunks]
            bt = cpool.tile([128, 1152], f16)
            nc.vector.tensor_copy(bt[:, 0:128], ident[:])
            for c in range(4):
                nc.vector.tensor_scalar(
                    bt[:, 128 + c * 128 : 256 + c * 128], ident[:],
                    zv[:, c : c + 1], None, op0=AO.mult,
                )
                nc.vector.tensor_scalar(
                    bt[:, 640 + c * 128 : 768 + c * 128], ident[:],
                    yv[:, c : c + 1], None, op0=AO.mult,
                )
            nc.sync.dma_start(out=bands.ap()[:, :], in_=bt[:])

    nc.compile()
    return nc


def _get_nc(opts=None):
    key = ("nc", tuple(sorted((opts or {}).items())))
    if key not in _CACHE:
        _CACHE[key] = _build(opts)
    return _CACHE[key]


def _in_maps(M, params, kinds):
    maps = []
    for d in range(D):
        maps.append(
            {
                "m_rows": np.ascontiguousarray(M[d * NR : (d + 1) * NR, :]),
                "params_s": np.ascontiguousarray(
                    params[d * EC : (d + 1) * EC].reshape(4, 128).T
                ),
                "kinds_s": np.ascontiguousarray(
                    kinds[d * EC : (d + 1) * EC].reshape(4, 128).T
                ),
            }
        )
    return maps


def kernel(M, params, kinds, _trace=False, _trace_kwargs=None, _opts=None):
    from concourse.bass_utils import run_bass_kernel_spmd

    M = np.ascontiguousarray(np.asarray(M, dtype=np.float32))
    params = np.ascontiguousarray(np.asarray(params, dtype=np.float32))
    kinds = np.ascontiguousarray(np.asarray(kinds, dtype=np.int32))
    assert M.shape == (N, E) and params.shape == (E,) and kinds.shape == (E,)

    nc = _get_nc(_opts)
    res = run_bass_kernel_spmd(
        nc,
        _in_maps(M, params, kinds),
        core_ids=list(range(D)),
        trace=_trace,
        **(_trace_kwargs or {}),
    )
    out = np.zeros((N + 2 * E, W), np.float32)
    for d in range(D):
        r = res.results[d]
        out[d * NR : (d + 1) * NR, 0:E] = r["kcl"]
        out[N : N + E, 2 * E + d * NR : 2 * E + (d + 1) * NR] = (
            r["mt"].reshape(128, 32, NR).transpose(1, 0, 2).reshape(E, NR)
        )
        b = r["bands"]
        eye3 = b[:, 0:128]
        zb3 = b[:, 128:640].reshape(128, 4, 128)
        yb3 = b[:, 640:1152].reshape(128, 4, 128)
        for c in range(4):
            g0 = d * EC + c * 128  # global elem index of band start
            out[N + g0 : N + g0 + 128, E + g0 : E + g0 + 128] = eye3
            out[N + E + g0 : N + E + g0 + 128, g0 : g0 + 128] = zb3[:, c, :]
            out[N + E + g0 : N + E + g0 + 128, E + g0 : E + g0 + 128] = yb3[:, c, :]
    if _trace:
        _CACHE["last_result"] = res
    return out
